# revision 2
# baseline (speedup 1.0000x reference)
"""BOW multi-hot regression kernel for trn2, 8 NeuronCores.

score[b, l] = sum_{v in distinct non-PAD tokens of doc b} W[l, v] + bias[l]

Strategy (V-sharded, single SPMD launch):
  - vocab padded to 50176 = 8 * 6272; core g owns rows [6272g, 6272(g+1)).
  - W.T is host-prepped to bf16, PAD column zeroed, bias appended as row 50175
    which is activated through two constant extra token slots per doc.
  - tokens are host-shifted per core (tok - 6272g) so chunk bases are
    compile-time constants; per core: DVE computes per-chunk masked int16
    indices; GPSIMD local_scatter builds the multi-hot bow [128 docs, 6272]
    bf16 per doc-tile (duplicate tokens overwrite the same cell with the same
    1.0 -> dedup for free); HWDGE xbar DMA transposes it to ktile layout
    [128 v, 49, 128 docs]; PE accumulates 49 matmuls per doc-tile into PSUM
    -> partial [1024, 512] f16 streamed to DRAM per doc-tile;
    one f16 ReduceScatter sums partials; core g outputs docs [128g, 128(g+1))
    as f16 (host casts to f32).
  - schedule: tok DMA owns HBM first (wt gated behind it, 7 chunks so early
    ktiles unblock early matmuls); doc-tile 0 chunk-0 indices computed in a
    small slice so the first scatter starts as early as possible; the
    ReduceScatter is a single op at the end (chunked RS serializes on the
    trigger engine and fights the transpose/collective serialization rule).
"""

import sys

sys.path.insert(0, "/opt/trn_rl_repo")

import numpy as np
import ml_dtypes

from concourse import bass, bacc, tile, mybir, bass_utils
from concourse.tile import add_dep_helper

# problem constants
T, B, V, L = 200, 1024, 50000, 512
PAD = 1
NCORES = 8
VP = 50176            # padded vocab, = NCORES * VC
VC = VP // NCORES     # 6272 vocab rows per core = KT * 128
KT = VC // 128        # 49 ktiles
CH = 1568             # local_scatter chunk width (1568 * 32 < 2**16)
NCH = VC // CH        # 4 chunks
TS = T + 2            # token slots per doc: 200 real + bias slot + filler
DT = B // 128         # 8 doc-tiles
BIAS_SLOT = VP - 1    # 50175
FILL_SLOT = VP - 2    # 50174
TOKW = DT * TS        # flat tok row: DT doc slices (host pre-shifted)
NWT = 7               # wt load chunks (7 ktiles each)

_cache = {}


def _build_nc():
    nc = bacc.Bacc("TRN2", target_bir_lowering=False, debug=False,
                   num_devices=NCORES)
    f32 = mybir.dt.float32
    bf16 = mybir.dt.bfloat16
    i32 = mybir.dt.int32
    i16 = mybir.dt.int16
    f16 = mybir.dt.float16
    Alu = mybir.AluOpType

    # tok row layout per partition p: [DT, TS] tokens of docs {dt*128+p},
    # already shifted by -core_base on host (so values are core-local)
    tok_d = nc.dram_tensor("tok", [128, TOKW], i32, kind="ExternalInput")
    wt_d = nc.dram_tensor("wt", [128, KT, 512], bf16, kind="ExternalInput")
    out_d = nc.dram_tensor("out", [128, 512], f16, kind="ExternalOutput")

    with tile.TileContext(nc) as tc:
        with tc.tile_pool(name="const", bufs=1) as cpool, \
             tc.tile_pool(name="work", bufs=3) as wpool, \
             tc.tile_pool(name="bow", bufs=4) as bpool, \
             tc.tile_pool(name="bowt", bufs=4) as btpool, \
             tc.tile_pool(name="psum", bufs=8, space="PSUM") as ppool, \
             tc.tile_pool(name="dram", bufs=1, space="DRAM") as dpool:

            tok_sb = cpool.tile([128, TOKW], i32, tag="tok")
            tok_dma = nc.sync.dma_start(out=tok_sb[:], in_=tok_d.ap())

            # wt in 7 ktile-ordered chunks, all gated behind tok so tok owns
            # the HBM first; finer chunks let early matmuls start before the
            # whole 6.4MB load lands
            wt_sb = cpool.tile([128, KT, 512], bf16, tag="wt")
            for j in range(NWT):
                k0, k1 = j * 7, (j + 1) * 7
                wdma = nc.scalar.dma_start(
                    out=wt_sb[:, k0:k1, :], in_=wt_d.ap()[:, k0:k1, :]
                )
                if j == 0:
                    add_dep_helper(wdma.ins, tok_dma.ins, sync=True,
                                   reason="tok DMA gates the whole pipeline")

            ones_sb = cpool.tile([128, TS], bf16, tag="ones")
            nc.vector.memset(ones_sb[:], 1.0)

            # chunk base constants [128, NCH, TS] (values c*CH), built once
            bases_sb = cpool.tile([128, NCH, TS], i32, tag="bases")
            for c in range(NCH):
                nc.vector.memset(bases_sb[:, c, :], c * CH)

            partial_sb = cpool.tile([128, DT, 512], f16, tag="partial")
            pd = dpool.tile([B, 512], f16, tag="pdram")
            pd_pm = pd[:].rearrange("(d p) l -> p d l", p=128)

            for dt in range(DT):
                # masked local chunk indices: for chunk c,
                # idx = tok_local - 1568*c  if in [0, 1568) else negative
                tokrep = (
                    tok_sb[:, dt * TS:(dt + 1) * TS]
                    .unsqueeze(1)
                    .broadcast_to((128, NCH, TS))
                )
                d_t = wpool.tile([128, NCH, TS], i32, tag="d")
                m_t = wpool.tile([128, NCH, TS], i32, tag="m")
                idx_t = wpool.tile([128, NCH, TS], i16, tag="idx")
                if dt == 0:
                    # per-chunk slices so chunk 0's scatter starts without
                    # waiting for the whole doc-tile's index math
                    for c in range(NCH):
                        nc.vector.tensor_tensor(
                            out=d_t[:, c:c + 1, :],
                            in0=tokrep[:, c:c + 1, :],
                            in1=bases_sb[:, c:c + 1, :],
                            op=Alu.subtract,
                        )
                        nc.vector.tensor_scalar(
                            out=d_t[:, c:c + 1, :], in0=d_t[:, c:c + 1, :],
                            scalar1=32767, scalar2=-1,
                            op0=Alu.min, op1=Alu.max,
                        )
                        nc.vector.tensor_scalar(
                            out=m_t[:, c:c + 1, :], in0=d_t[:, c:c + 1, :],
                            scalar1=CH, scalar2=-32768,
                            op0=Alu.is_ge, op1=Alu.mult,
                        )
                        nc.vector.tensor_tensor(
                            out=idx_t[:, c, :],
                            in0=d_t[:, c, :], in1=m_t[:, c, :],
                            op=Alu.add,
                        )
                else:
                    nc.vector.tensor_tensor(
                        out=d_t[:], in0=tokrep, in1=bases_sb[:],
                        op=Alu.subtract,
                    )
                    nc.vector.tensor_scalar(
                        out=d_t[:], in0=d_t[:],
                        scalar1=32767, scalar2=-1, op0=Alu.min, op1=Alu.max,
                    )
                    nc.vector.tensor_scalar(
                        out=m_t[:], in0=d_t[:],
                        scalar1=CH, scalar2=-32768, op0=Alu.is_ge, op1=Alu.mult,
                    )
                    nc.vector.tensor_tensor(
                        out=idx_t[:], in0=d_t[:], in1=m_t[:], op=Alu.add,
                    )

                bow_t = bpool.tile([128, VC], bf16, tag="bow")
                for c in range(NCH):
                    nc.gpsimd.local_scatter(
                        bow_t[:, c * CH:(c + 1) * CH],
                        ones_sb[:],
                        idx_t[:, c, :],
                        channels=128,
                        num_elems=CH,
                        num_idxs=TS,
                    )

                bowt_t = btpool.tile([128, KT, 128], bf16, tag="bowt")
                ps = ppool.tile([128, 512], f32, tag="ps")
                k0 = 0
                for c in range(NCH):
                    k1 = ((c + 1) * CH) // 128 if c < NCH - 1 else KT
                    nc.sync.dma_start(
                        out=bowt_t[:, k0:k1, :],
                        in_=bow_t[:, k0 * 128:k1 * 128],
                        transpose=True,
                    )
                    for k in range(k0, k1):
                        nc.tensor.matmul(
                            out=ps[:],
                            lhsT=bowt_t[:, k, :],
                            rhs=wt_sb[:, k, :],
                            start=(k == 0),
                            stop=(k == KT - 1),
                        )
                    k0 = k1
                nc.vector.tensor_copy(out=partial_sb[:, dt, :], in_=ps[:])
                # stream this doc-tile's partial to DRAM so the final
                # ReduceScatter only waits on the last one
                nc.scalar.dma_start(
                    out=pd_pm[:, dt, :], in_=partial_sb[:, dt, :]
                )

            # single f16 ReduceScatter over the full [1024, 512] partial;
            # core g receives docs [128g, 128g+128)
            rs = dpool.tile([128, 512], f16, tag="rsout")
            nc.gpsimd.collective_compute(
                "ReduceScatter",
                mybir.AluOpType.add,
                replica_groups=[list(range(NCORES))],
                ins=[pd.opt()],
                outs=[rs.opt()],
            )
            nc.sync.dma_start(out=out_d.ap(), in_=rs[:])

    nc.compile()
    return nc


def _host_prep(text, W, b):
    # tokens: [T, B] -> [B, T] int32, append bias + filler slots
    tok = np.ascontiguousarray(text.T).astype(np.int32)          # [B, T]
    extra = np.empty((B, 2), np.int32)
    extra[:, 0] = BIAS_SLOT
    extra[:, 1] = FILL_SLOT
    tok = np.concatenate([tok, extra], axis=1)                   # [B, TS]
    # partition-major pack: row p = docs {dt*128+p for dt in range(DT)}
    tok_pm = np.ascontiguousarray(
        tok.reshape(DT, 128, TS).transpose(1, 0, 2)
    ).reshape(128, DT * TS)

    # weights: Wt [VP, 512] bf16, PAD column zeroed, bias row appended
    Wt = np.zeros((VP, L), np.float32)
    Wt[:V] = W.T
    Wt[PAD] = 0.0
    Wt[BIAS_SLOT] = b
    Wt = Wt.astype(ml_dtypes.bfloat16)

    in_maps = []
    for g in range(NCORES):
        tok_g = tok_pm - np.int32(g * VC)                        # [128, TOKW]
        wt_g = np.ascontiguousarray(
            Wt[g * VC:(g + 1) * VC].reshape(KT, 128, L).transpose(1, 0, 2)
        )                                                        # [128, KT, 512]
        in_maps.append({"tok": tok_g, "wt": wt_g})
    return in_maps


def kernel(text, W, b, trace=False, trace_kwargs=None):
    if "nc" not in _cache:
        _cache["nc"] = _build_nc()
    nc = _cache["nc"]
    in_maps = _host_prep(np.asarray(text), np.asarray(W), np.asarray(b))
    res = bass_utils.run_bass_kernel_spmd(
        nc, in_maps, core_ids=list(range(NCORES)),
        trace=trace, **(trace_kwargs or {}),
    )
    _cache["last_results"] = res
    out = np.empty((B, L), np.float32)
    for g in range(NCORES):
        out[g * 128:(g + 1) * 128] = res.results[g]["out"].astype(np.float32)
    return out


# revision 5
# speedup vs baseline: 1.0911x; 1.0911x over previous
"""BOW multi-hot regression kernel for trn2, 8 NeuronCores.

score[b, l] = sum_{v in distinct non-PAD tokens of doc b} W[l, v] + bias[l]

Strategy (V-sharded, single SPMD launch):
  - vocab padded to 50176 = 8 * 6272; core g owns rows [6272g, 6272(g+1)).
  - W.T is host-prepped to bf16, PAD column zeroed, bias appended as row 50175
    which is activated through two constant extra token slots per doc.
  - tokens are host-shifted per core (tok - 6272g) so chunk bases are
    compile-time constants; per core: DVE computes per-chunk masked int16
    indices; GPSIMD local_scatter builds the multi-hot bow [128 docs, 6272]
    bf16 per doc-tile (duplicate tokens overwrite the same cell with the same
    1.0 -> dedup for free); HWDGE xbar DMA transposes it to ktile layout
    [128 v, 49, 128 docs]; PE accumulates 49 matmuls per doc-tile into PSUM
    -> partial [1024, 512] f16 streamed to DRAM per doc-tile;
    one f16 ReduceScatter sums partials; core g outputs docs [128g, 128(g+1))
    as f16 (host casts to f32).
  - schedule: tok DMA owns HBM first (wt gated behind it, 7 chunks so early
    ktiles unblock early matmuls); doc-tile 0 chunk-0 indices computed in a
    small slice so the first scatter starts as early as possible; the
    ReduceScatter is a single op at the end (chunked RS serializes on the
    trigger engine and fights the transpose/collective serialization rule).
"""

import sys

sys.path.insert(0, "/opt/trn_rl_repo")

import numpy as np
import ml_dtypes

from concourse import bass, bacc, tile, mybir, bass_utils
from concourse.tile import add_dep_helper

# problem constants
T, B, V, L = 200, 1024, 50000, 512
PAD = 1
NCORES = 8
VP = 50176            # padded vocab, = NCORES * VC
VC = VP // NCORES     # 6272 vocab rows per core = KT * 128
KT = VC // 128        # 49 ktiles
CH = 1568             # local_scatter chunk width (1568 * 32 < 2**16)
NCH = VC // CH        # 4 chunks
TS = T + 2            # token slots per doc: 200 real + bias slot + filler
DT = B // 128         # 8 doc-tiles
BIAS_SLOT = VP - 1    # 50175
FILL_SLOT = VP - 2    # 50174
TOKW = DT * TS        # flat tok row: DT doc slices (host pre-shifted)
NWT = 7               # wt load chunks (7 ktiles each)

_cache = {}


def _build_nc():
    nc = bacc.Bacc("TRN2", target_bir_lowering=False, debug=False,
                   num_devices=NCORES)
    f32 = mybir.dt.float32
    bf16 = mybir.dt.bfloat16
    i32 = mybir.dt.int32
    i16 = mybir.dt.int16
    f16 = mybir.dt.float16
    Alu = mybir.AluOpType

    # tok row layout per partition p: [DT, TS] tokens of docs {dt*128+p},
    # already shifted by -core_base on host (so values are core-local)
    tok_d = nc.dram_tensor("tok", [128, TOKW], i32, kind="ExternalInput")
    wt_d = nc.dram_tensor("wt", [128, KT, 512], bf16, kind="ExternalInput")
    out_d = nc.dram_tensor("out", [128, 512], f16, kind="ExternalOutput")

    with tile.TileContext(nc) as tc:
        with tc.tile_pool(name="const", bufs=1) as cpool, \
             tc.tile_pool(name="work", bufs=3) as wpool, \
             tc.tile_pool(name="bow", bufs=4) as bpool, \
             tc.tile_pool(name="bowt", bufs=4) as btpool, \
             tc.tile_pool(name="psum", bufs=8, space="PSUM") as ppool, \
             tc.tile_pool(name="dram", bufs=1, space="DRAM") as dpool:

            tok_sb = cpool.tile([128, TOKW], i32, tag="tok")
            tok_dma = nc.sync.dma_start(out=tok_sb[:], in_=tok_d.ap())

            # wt in 4 ktile-ordered chunks, all gated behind tok so tok owns
            # the HBM first; chunked so early matmuls start before the whole
            # 6.4MB load lands (too many chunks makes the scheduler chain
            # them behind transposes via completion-sem recycling)
            wt_sb = cpool.tile([128, KT, 512], bf16, tag="wt")
            for k0, k1 in ((0, 8), (8, 21), (21, 34), (34, KT)):
                wdma = nc.scalar.dma_start(
                    out=wt_sb[:, k0:k1, :], in_=wt_d.ap()[:, k0:k1, :]
                )
                add_dep_helper(wdma.ins, tok_dma.ins, sync=True,
                               reason="tok DMA gates the whole pipeline")

            ones_sb = cpool.tile([128, TS], bf16, tag="ones")
            nc.vector.memset(ones_sb[:], 1.0)

            # dummy scatter with an all-zero index tile: triggers the Q7
            # local_scatter library IRAM load during the preamble and absorbs
            # the expensive first-call warmup off the critical path
            zidx = cpool.tile([128, TS], i16, tag="zidx")
            nc.vector.memset(zidx[:], 0)
            scr = cpool.tile([128, CH], bf16, tag="scr")
            nc.gpsimd.local_scatter(
                scr[:], ones_sb[:], zidx[:], channels=128, num_elems=CH,
                num_idxs=TS,
            )

            # chunk base constants [128, NCH, TS] (values c*CH), built once
            bases_sb = cpool.tile([128, NCH, TS], i32, tag="bases")
            for c in range(NCH):
                nc.vector.memset(bases_sb[:, c, :], c * CH)

            partial_sb = cpool.tile([128, DT, 512], f16, tag="partial")
            pd = dpool.tile([B, 512], f16, tag="pdram")
            pd_pm = pd[:].rearrange("(d p) l -> p d l", p=128)

            for dt in range(DT):
                # masked local chunk indices: for chunk c,
                # idx = tok_local - 1568*c  if in [0, 1568) else negative
                tokrep = (
                    tok_sb[:, dt * TS:(dt + 1) * TS]
                    .unsqueeze(1)
                    .broadcast_to((128, NCH, TS))
                )
                d_t = wpool.tile([128, NCH, TS], i32, tag="d")
                m_t = wpool.tile([128, NCH, TS], i32, tag="m")
                idx_t = wpool.tile([128, NCH, TS], i16, tag="idx")
                if dt == 0:
                    # per-chunk slices so chunk 0's scatter starts without
                    # waiting for the whole doc-tile's index math
                    for c in range(NCH):
                        nc.vector.tensor_tensor(
                            out=d_t[:, c:c + 1, :],
                            in0=tokrep[:, c:c + 1, :],
                            in1=bases_sb[:, c:c + 1, :],
                            op=Alu.subtract,
                        )
                        nc.vector.tensor_scalar(
                            out=d_t[:, c:c + 1, :], in0=d_t[:, c:c + 1, :],
                            scalar1=32767, scalar2=-1,
                            op0=Alu.min, op1=Alu.max,
                        )
                        nc.vector.tensor_scalar(
                            out=m_t[:, c:c + 1, :], in0=d_t[:, c:c + 1, :],
                            scalar1=CH, scalar2=-32768,
                            op0=Alu.is_ge, op1=Alu.mult,
                        )
                        nc.vector.tensor_tensor(
                            out=idx_t[:, c, :],
                            in0=d_t[:, c, :], in1=m_t[:, c, :],
                            op=Alu.add,
                        )
                else:
                    nc.vector.tensor_tensor(
                        out=d_t[:], in0=tokrep, in1=bases_sb[:],
                        op=Alu.subtract,
                    )
                    nc.vector.tensor_scalar(
                        out=d_t[:], in0=d_t[:],
                        scalar1=32767, scalar2=-1, op0=Alu.min, op1=Alu.max,
                    )
                    nc.vector.tensor_scalar(
                        out=m_t[:], in0=d_t[:],
                        scalar1=CH, scalar2=-32768, op0=Alu.is_ge, op1=Alu.mult,
                    )
                    nc.vector.tensor_tensor(
                        out=idx_t[:], in0=d_t[:], in1=m_t[:], op=Alu.add,
                    )

                bow_t = bpool.tile([128, VC], bf16, tag="bow")
                for c in range(NCH):
                    nc.gpsimd.local_scatter(
                        bow_t[:, c * CH:(c + 1) * CH],
                        ones_sb[:],
                        idx_t[:, c, :],
                        channels=128,
                        num_elems=CH,
                        num_idxs=TS,
                    )

                bowt_t = btpool.tile([128, KT, 128], bf16, tag="bowt")
                ps = ppool.tile([128, 512], f32, tag="ps")
                k0 = 0
                for c in range(NCH):
                    k1 = ((c + 1) * CH) // 128 if c < NCH - 1 else KT
                    nc.sync.dma_start(
                        out=bowt_t[:, k0:k1, :],
                        in_=bow_t[:, k0 * 128:k1 * 128],
                        transpose=True,
                    )
                    for k in range(k0, k1):
                        nc.tensor.matmul(
                            out=ps[:],
                            lhsT=bowt_t[:, k, :],
                            rhs=wt_sb[:, k, :],
                            start=(k == 0),
                            stop=(k == KT - 1),
                        )
                    k0 = k1
                nc.vector.tensor_copy(out=partial_sb[:, dt, :], in_=ps[:])
                # stream this doc-tile's partial to DRAM so the final
                # ReduceScatter only waits on the last one
                nc.scalar.dma_start(
                    out=pd_pm[:, dt, :], in_=partial_sb[:, dt, :]
                )

            # AllToAll (no CCE-reduce penalty, ~2x the wire rate of RS):
            # core g receives every core's partial for docs [128g, 128g+128),
            # then sums the 8 contributions locally on DVE (f16 tree)
            pd2 = dpool.tile([B, 512], f16, tag="a2aout")
            nc.gpsimd.collective_compute(
                "AllToAll",
                mybir.AluOpType.bypass,
                replica_groups=[list(range(NCORES))],
                ins=[pd.opt()],
                outs=[pd2.opt()],
            )
            rbuf = cpool.tile([128, DT, 512], f16, tag="rbuf")
            nc.sync.dma_start(
                out=rbuf[:], in_=pd2[:].rearrange("(d p) l -> p d l", p=128)
            )
            rtmp = cpool.tile([128, 4, 512], f16, tag="rtmp")
            for h in range(4):
                nc.vector.tensor_tensor(
                    out=rtmp[:, h, :], in0=rbuf[:, 2 * h, :],
                    in1=rbuf[:, 2 * h + 1, :], op=Alu.add,
                )
            for h in range(2):
                nc.vector.tensor_tensor(
                    out=rtmp[:, 2 * h, :], in0=rtmp[:, 2 * h, :],
                    in1=rtmp[:, 2 * h + 1, :], op=Alu.add,
                )
            rsum = cpool.tile([128, 512], f16, tag="rsum")
            nc.vector.tensor_tensor(
                out=rsum[:], in0=rtmp[:, 0, :], in1=rtmp[:, 2, :], op=Alu.add,
            )
            nc.sync.dma_start(out=out_d.ap(), in_=rsum[:])

    nc.compile()
    return nc


def _host_prep(text, W, b):
    # tokens: [T, B] -> [B, T] int32, append bias + filler slots
    tok = np.ascontiguousarray(text.T).astype(np.int32)          # [B, T]
    extra = np.empty((B, 2), np.int32)
    extra[:, 0] = BIAS_SLOT
    extra[:, 1] = FILL_SLOT
    tok = np.concatenate([tok, extra], axis=1)                   # [B, TS]
    # partition-major pack: row p = docs {dt*128+p for dt in range(DT)}
    tok_pm = np.ascontiguousarray(
        tok.reshape(DT, 128, TS).transpose(1, 0, 2)
    ).reshape(128, DT * TS)

    # weights: Wt [VP, 512] bf16, PAD column zeroed, bias row appended
    Wt = np.zeros((VP, L), np.float32)
    Wt[:V] = W.T
    Wt[PAD] = 0.0
    Wt[BIAS_SLOT] = b
    Wt = Wt.astype(ml_dtypes.bfloat16)

    in_maps = []
    for g in range(NCORES):
        tok_g = tok_pm - np.int32(g * VC)                        # [128, TOKW]
        wt_g = np.ascontiguousarray(
            Wt[g * VC:(g + 1) * VC].reshape(KT, 128, L).transpose(1, 0, 2)
        )                                                        # [128, KT, 512]
        in_maps.append({"tok": tok_g, "wt": wt_g})
    return in_maps


def kernel(text, W, b, trace=False, trace_kwargs=None):
    if "nc" not in _cache:
        _cache["nc"] = _build_nc()
    nc = _cache["nc"]
    in_maps = _host_prep(np.asarray(text), np.asarray(W), np.asarray(b))
    res = bass_utils.run_bass_kernel_spmd(
        nc, in_maps, core_ids=list(range(NCORES)),
        trace=trace, **(trace_kwargs or {}),
    )
    _cache["last_results"] = res
    out = np.empty((B, L), np.float32)
    for g in range(NCORES):
        out[g * 128:(g + 1) * 128] = res.results[g]["out"].astype(np.float32)
    return out
